# revision 53
# baseline (speedup 1.0000x reference)
"""GNN (3x SAGEConv mean-aggr + attention pooling + MLP) on 8 Trainium2 cores.

Data-parallel over graphs: each core owns 256 consecutive graphs (a
contiguous node range). Edge aggregation gathers source-node rows from a
replicated bf16 node table (dma_gather, int16 idxs over mod-4 strided
views, 128B payload / 256B stride rows), then scatters into per-block
PSUM accumulators via one-hot matmuls. Node features are exchanged
between layers with an on-device AllGather.
"""
import sys
sys.path.insert(0, '/opt/trn_rl_repo')
import hashlib
import numpy as np

NG = 2048
NC = 8
GPC = NG // NC            # graphs per core = 256
P = 128
GB = 12                   # node blocks per gather group
MAXCH = 30                # chunks per dma_gather call (7680 idxs, 30.7KB Q7 idx
                          # scratch; needs single_packet=False: the evt-accel
                          # doorbell in single-packet mode wedges past 1024 idxs
SW = 4                    # chunks per is_equal op

_CACHE = {}
_PATCHED = [False]


def _patch_dma_gather_assert():
    """Allow 128B gather payloads (row stride must still be 256B-aligned)."""
    if _PATCHED[0]:
        return
    import inspect, textwrap
    import concourse.bass as cb
    src = textwrap.dedent(inspect.getsource(cb.BassGpSimd.dma_gather))
    old = """    assert (
        elem_size_bytes > 0 and elem_size_bytes % 256 == 0
    )  # transpose restriction"""
    assert old in src, "dma_gather source changed; update patch"
    src = src.replace(old, "    assert elem_size_bytes > 0")
    src = ("import concourse.bass\n"
           "from concourse.bass import *\n"
           "from concourse.bass import ap_utils\n" + src)
    ns = {}
    exec(src, vars(cb), ns)
    cb.BassGpSimd.dma_gather = ns["dma_gather"]
    _PATCHED[0] = True


def _preprocess(edge_index, batch_index):
    src = np.asarray(edge_index[0], np.int64)
    dst = np.asarray(edge_index[1], np.int64)
    batch = np.asarray(batch_index, np.int64)
    n_nodes = batch.size

    node_start = np.searchsorted(batch, np.arange(NC) * GPC, side='left')
    node_start = np.append(node_start, n_nodes)
    Mc = np.diff(node_start)
    NB = int(np.ceil(Mc.max() / P))
    CAP = NB * P
    V = NC * CAP

    core_of = np.repeat(np.arange(NC), Mc)
    lid = np.arange(n_nodes) - node_start[core_of]
    g = core_of * CAP + lid                      # padded-global id

    cnt = np.bincount(dst, minlength=n_nodes)
    inv = (1.0 / np.maximum(cnt, 1)).astype(np.float32)

    ec = core_of[dst]
    ld = dst - node_start[ec]
    eb = ld >> 7
    slot = (ld & 127).astype(np.float32)
    gs = g[src]
    er = (gs & 3).astype(np.int64)
    idxv = (gs >> 2)
    assert idxv.max() < 32768

    counts = np.bincount((ec * NB + eb) * 4 + er, minlength=NC * NB * 4)
    counts = counts.reshape(NC, NB, 4)
    Kmax = np.ceil(counts.max(axis=0) / P).astype(np.int64)   # [NB, 4]

    # Two chunk orderings over the same chunks:
    #  x-order (gather): for grp, for r, for b in grp, for k  -> contiguous
    #    per (grp, r) so each dma_gather call is one idx column range
    #  d-order (S build): for grp, for b, for r, for k -> contiguous per
    #    block so one is_equal covers up to SW chunks of a block
    off_x = np.zeros((NB, 4), np.int64)
    off_d = np.zeros((NB, 4), np.int64)
    groups = []      # (b0, b1, gstart, calls[(r, xs, xe)])
    ct = 0
    for b0 in range(0, NB, GB):
        b1 = min(b0 + GB, NB)
        gstart = ct
        calls = []
        for r in range(4):
            cs = ct
            for b in range(b0, b1):
                off_x[b, r] = ct
                ct += Kmax[b, r]
            for sub in range(cs, ct, MAXCH):
                calls.append((r, sub, min(sub + MAXCH, ct)))
        groups.append((b0, b1, gstart, calls))
        dd = gstart
        for b in range(b0, b1):
            for r in range(4):
                off_d[b, r] = dd
                dd += Kmax[b, r]
        assert dd == ct
    CT = ct

    idx16 = np.zeros((NC, 16, CT * 8), np.int16)
    dstrel = np.full((NC, P, CT), -1.0, np.float32)
    for c in range(NC):
        m = ec == c
        eb_c = eb[m]
        er_c = er[m]
        # idxv innermost: ascending gather addresses within each chunk run
        order = np.lexsort((idxv[m], er_c, eb_c))
        ebo = eb_c[order]
        ero = er_c[order]
        sk = ebo * 4 + ero
        startmask = np.r_[True, sk[1:] != sk[:-1]]
        grp_start_pos = np.flatnonzero(startmask)
        grp_id = np.cumsum(startmask) - 1
        pos = np.arange(sk.size) - grp_start_pos[grp_id]
        chw = pos >> 7
        p = pos & 127
        chunk_x = off_x[ebo, ero] + chw
        chunk_d = off_d[ebo, ero] + chw
        idx16[c][p & 15, chunk_x * 8 + (p >> 4)] = idxv[m][order].astype(np.int16)
        dstrel[c][p, chunk_d] = slot[m][order]
    idx16 = np.ascontiguousarray(np.tile(idx16, (1, 8, 1)))   # [NC,128,CT*8]

    invb = np.ones((NC, 64, CAP), np.float32)
    brel = np.full((NC, P, NB), -1.0, np.float32)
    for c in range(NC):
        M = Mc[c]
        invb[c, :, :M] = inv[node_start[c]:node_start[c + 1]][None, :]
        br = (batch[node_start[c]:node_start[c + 1]] - c * GPC).astype(np.float32)
        full = np.full(CAP, -1.0, np.float32)
        full[:M] = br
        brel[c] = full.reshape(NB, P).T

    # per-block S-build schedule: list of (d0, w, [x-order chunk ids])
    sbuild = []
    for b in range(NB):
        segs = []
        cols = []
        for r in range(4):
            for k in range(int(Kmax[b, r])):
                cols.append(int(off_x[b, r]) + k)
        d0 = int(off_d[b, 0])
        L = len(cols)
        j = 0
        while j < L:
            w = min(SW, L - j)
            segs.append((d0 + j, w, cols[j:j + w]))
            j += w
        sbuild.append(segs)

    return dict(
        node_start=node_start, Mc=Mc, NB=NB, CAP=CAP, V=V, g=g,
        Kmax=Kmax, groups=groups, CT=CT, sbuild=sbuild,
        idx16=idx16, dstrel=dstrel, invb=invb, brel=brel,
    )


def _build_nc(meta):
    import concourse.bacc as bacc
    import concourse.tile as tile
    from concourse import mybir

    _patch_dma_gather_assert()

    NB, CAP, V, CT = meta['NB'], meta['CAP'], meta['V'], meta['CT']
    groups, sbuild = meta['groups'], meta['sbuild']
    dt = mybir.dt.float32
    bt = mybir.dt.bfloat16
    AT = mybir.ActivationFunctionType
    OP = mybir.AluOpType

    nc = bacc.Bacc("TRN2", debug=False, num_swdge_queues=4)

    t_table1 = nc.dram_tensor("table1", [V, 64], bt, kind="ExternalInput")
    t_xTb = nc.dram_tensor("xTb", [NB, 64, P], dt, kind="ExternalInput")
    t_idx = nc.dram_tensor("idx16", [P, CT * 8], mybir.dt.int16, kind="ExternalInput")
    t_dst = nc.dram_tensor("dstrel", [P, CT], bt, kind="ExternalInput")
    t_invb = nc.dram_tensor("invb", [64, CAP], dt, kind="ExternalInput")
    t_brel = nc.dram_tensor("brel", [P, NB], dt, kind="ExternalInput")
    t_iota128 = nc.dram_tensor("iota128", [P, P], bt, kind="ExternalInput")
    t_iota256 = nc.dram_tensor("iota256", [P, 256], dt, kind="ExternalInput")
    t_idgw = nc.dram_tensor("idgw", [64, 65], dt, kind="ExternalInput")
    t_ones64 = nc.dram_tensor("ones64", [1, 64], dt, kind="ExternalInput")
    wnames = ["w1l", "w1r", "w2l", "w2r", "w3l", "w3r", "lin1_w"]
    t_w = {n: nc.dram_tensor(n, [64, 64], dt, kind="ExternalInput") for n in wnames}
    t_b = {n: nc.dram_tensor(n, [64, 1], dt, kind="ExternalInput")
           for n in ["b1l", "b2l", "b3l", "lin1_b"]}
    t_gb = nc.dram_tensor("gate_b", [P, 1], dt, kind="ExternalInput")
    t_l2w = nc.dram_tensor("lin2_w", [64, 1], dt, kind="ExternalInput")
    t_l2b = nc.dram_tensor("lin2_b", [1, 1], dt, kind="ExternalInput")
    t_y = nc.dram_tensor("y", [1, GPC], dt, kind="ExternalOutput")

    with tile.TileContext(nc) as tc:
        with tc.tile_pool(name="const", bufs=1) as cp, \
             tc.tile_pool(name="xg", bufs=2) as xgp, \
             tc.tile_pool(name="s", bufs=4) as sp, \
             tc.tile_pool(name="sp2", bufs=2) as sp2, \
             tc.tile_pool(name="blk", bufs=3) as bp, \
             tc.tile_pool(name="grp", bufs=2) as gp, \
             tc.tile_pool(name="ep", bufs=1) as ep, \
             tc.tile_pool(name="psA", bufs=2, space="PSUM") as psA, \
             tc.tile_pool(name="psB", bufs=1, space="PSUM") as psB, \
             tc.tile_pool(name="dram", bufs=1, space="DRAM") as dp:

            def load_const(name, tsrc, shape, dtype=dt):
                t = cp.tile(shape, dtype, name=name, tag=name)
                nc.sync.dma_start(out=t[:], in_=tsrc[:])
                return t

            iota128 = load_const("iota128", t_iota128, [P, P], bt)
            iota256 = load_const("iota256", t_iota256, [P, 256])
            idgw = load_const("idgw", t_idgw, [64, 65])
            ones64 = load_const("ones64", t_ones64, [1, 64])
            w_t = {n: load_const(n, t_w[n], [64, 64]) for n in wnames}
            b_t = {n: load_const(n, t_b[n], [64, 1]) for n in t_b}
            gb_t = load_const("gate_b", t_gb, [P, 1])
            l2w_t = load_const("lin2_w", t_l2w, [64, 1])
            l2b_t = load_const("lin2_b", t_l2b, [1, 1])
            idx_t = load_const("idx16", t_idx, [P, CT * 8], mybir.dt.int16)
            dst_t = load_const("dstrel", t_dst, [P, CT], bt)
            brel_t = load_const("brel", t_brel, [P, NB])

            hT_dram = [dp.tile([NB, 64, P], dt, name=f"hTd{l}", tag=f"hT{l}")
                       for l in range(2)]
            ag_in = [dp.tile([CAP, 64], bt, name=f"agin{l}", tag=f"agin{l}")
                     for l in range(2)]
            ag_out = [dp.tile([V, 64], bt, name=f"agout{l}", tag=f"agout{l}",
                              addr_space="Shared")
                      for l in range(2)]

            lw = [w_t["w1l"], w_t["w2l"], w_t["w3l"]]
            rw = [w_t["w1r"], w_t["w2r"], w_t["w3r"]]
            lb = [b_t["b1l"], b_t["b2l"], b_t["b3l"]]
            pool_ps = psB.tile([65, 256], dt, space="PSUM", tag="pool")

            import os as _os
            _amp = int(_os.environ.get("BASS_AMP", "1"))
            qrr = [0]
            for rep in range(_amp):
              agl = ag_out if rep == 0 else [
                  dp.tile([V, 64], bt, name=f"agoutr{rep}_{l}",
                          tag=f"agoutr{rep}_{l}", addr_space="Shared")
                  for l in range(2)]
              for layer in range(3):
                table = t_table1 if layer == 0 else agl[layer - 1]
                src_x = t_xTb if layer == 0 else hT_dram[layer - 1]
                for (b0, b1, gstart, calls) in groups:
                    nblk = b1 - b0
                    nch_grp = max(1, (calls[-1][2] - gstart) if calls else 1)
                    xg = xgp.tile([P, nch_grp, 64], bt, tag="xg")
                    for (r, xs, xe) in calls:
                        nch = xe - xs
                        nc.gpsimd.dma_gather(
                            xg[:, xs - gstart:xe - gstart, :],
                            table[r::4, :],
                            idx_t[:, xs * 8:xe * 8],
                            nch * P, nch * P, 64,
                            elem_step=4 * 64,
                            single_packet=False,
                            queue_num=qrr[0] % 4,
                        )
                        qrr[0] += 1
                    xTb_g = gp.tile([64, nblk, P], dt, tag="xTb_g")
                    nc.sync.dma_start(
                        out=xTb_g[:],
                        in_=src_x[b0:b1].rearrange("g f p -> f g p"))
                    invb_g = gp.tile([64, nblk, P], dt, tag="invb_g")
                    nc.sync.dma_start(
                        out=invb_g[:],
                        in_=t_invb[:, b0 * P:b1 * P].rearrange(
                            "f (g p) -> f g p", p=P))
                    hT_g = gp.tile([64, nblk, P], dt, tag="hT_g")
                    hnm_g = gp.tile([P, nblk, 64], bt, name="hnm_g",
                                    tag="hnm_g") if layer < 2 else None

                    for b in range(b0, b1):
                        j = b - b0
                        segs = sbuild[b]
                        mean_t = bp.tile([64, P], dt, tag="mean")
                        if not segs:
                            nc.vector.memset(mean_t[:], 0.0)
                        else:
                            msg_ps = psA.tile([64, P], dt, space="PSUM", tag="msg")
                            n_mm = sum(w for (_, w, _) in segs)
                            mm = 0
                            for (d0, w, xcols) in segs:
                                S4 = sp.tile([P, SW, P], bt, tag="S")
                                nc.vector.tensor_tensor(
                                    out=S4[:, 0:w, :],
                                    in0=dst_t[:, d0:d0 + w].to_broadcast([P, w, P]),
                                    in1=iota128[:].rearrange(
                                        "p (a q) -> p a q", a=1).to_broadcast([P, w, P]),
                                    op=OP.is_equal,
                                )
                                for t in range(w):
                                    nc.tensor.matmul(
                                        msg_ps[:],
                                        lhsT=xg[:, xcols[t] - gstart, :],
                                        rhs=S4[:, t, :],
                                        start=(mm == 0), stop=(mm == n_mm - 1),
                                    )
                                    mm += 1
                            nc.vector.tensor_tensor(
                                out=mean_t[:], in0=msg_ps[:],
                                in1=invb_g[:, j, :], op=OP.mult)

                        out_ps = psA.tile([64, P], dt, space="PSUM", tag="out")
                        nc.tensor.matmul(out_ps[:], lhsT=lw[layer][:], rhs=mean_t[:],
                                         start=True, stop=False)
                        nc.tensor.matmul(out_ps[:], lhsT=rw[layer][:],
                                         rhs=xTb_g[:, j, :], start=False, stop=True)
                        nc.scalar.activation(hT_g[:, j, :], out_ps[:], AT.Relu,
                                             bias=lb[layer][:], scale=1.0)

                        if layer < 2:
                            tr_ps = psA.tile([P, 64], dt, space="PSUM", tag="tr")
                            nc.tensor.matmul(tr_ps[:], lhsT=hT_g[:, j, :],
                                             rhs=idgw[:, 0:64], start=True,
                                             stop=True)
                            nc.vector.tensor_copy(out=hnm_g[:, j, :], in_=tr_ps[:])
                        else:
                            # transpose + gate in one matmul: idgw = [I64 | gw]
                            tg_ps = psA.tile([P, 65], dt, space="PSUM", tag="tr")
                            nc.tensor.matmul(tg_ps[:], lhsT=hT_g[:, j, :],
                                             rhs=idgw[:], start=True, stop=True)
                            e_col = bp.tile([P, 1], dt, tag="ecol")
                            nc.scalar.activation(e_col[:], tg_ps[:, 64:65], AT.Exp,
                                                 bias=gb_t[:], scale=1.0)
                            eh = bp.tile([P, 65], dt, tag="eh")
                            nc.scalar.activation(eh[:, 0:64], tg_ps[:, 0:64],
                                                 AT.Copy, scale=e_col[:])
                            nc.vector.tensor_copy(out=eh[:, 64:65], in_=e_col[:])
                            Sp = sp2.tile([P, 256], dt, tag="Sp")
                            nc.vector.tensor_tensor(
                                out=Sp[:],
                                in0=brel_t[:, b:b + 1].to_broadcast([P, 256]),
                                in1=iota256[:], op=OP.is_equal)
                            nc.tensor.matmul(pool_ps[:], lhsT=eh[:], rhs=Sp[:],
                                             start=(b == 0), stop=(b == NB - 1))

                    if layer < 2:
                        nc.sync.dma_start(
                            out=hT_dram[layer][b0:b1].rearrange("g f p -> f g p"),
                            in_=hT_g[:])
                        nc.sync.dma_start(
                            out=ag_in[layer][b0 * P:b1 * P, :].rearrange(
                                "(g p) f -> p g f", p=P),
                            in_=hnm_g[:])

                if layer < 2:
                    nc.gpsimd.collective_compute(
                        "AllGather",
                        mybir.AluOpType.bypass,
                        replica_groups=[list(range(NC))],
                        ins=[ag_in[layer].opt()],
                        outs=[agl[layer].opt()],
                    )

            # ---- MLP head on pooled [65, 256] ----
            numT = ep.tile([64, 256], dt, tag="numT")
            nc.vector.tensor_copy(out=numT[:], in_=pool_ps[0:64, :])
            den = ep.tile([1, 256], dt, tag="den")
            nc.vector.tensor_scalar_max(den[:], pool_ps[64:65, :], 1e-30)
            dinv = ep.tile([1, 256], dt, tag="dinv")
            nc.vector.reciprocal(dinv[:], den[:])
            dinvb_ps = psB.tile([64, 256], dt, space="PSUM", tag="big")
            nc.tensor.matmul(dinvb_ps[:], lhsT=ones64[:], rhs=dinv[:],
                             start=True, stop=True)
            gT = ep.tile([64, 256], dt, tag="gT")
            nc.vector.tensor_tensor(out=gT[:], in0=numT[:], in1=dinvb_ps[:],
                                    op=OP.mult)
            z1_ps = psB.tile([64, 256], dt, space="PSUM", tag="big")
            nc.tensor.matmul(z1_ps[:], lhsT=w_t["lin1_w"][:], rhs=gT[:],
                             start=True, stop=True)
            z1 = ep.tile([64, 256], dt, tag="z1")
            nc.scalar.activation(z1[:], z1_ps[:], AT.Relu,
                                 bias=b_t["lin1_b"][:], scale=1.0)
            y_ps = psB.tile([1, 256], dt, space="PSUM", tag="big")
            nc.tensor.matmul(y_ps[:], lhsT=l2w_t[:], rhs=z1[:],
                             start=True, stop=True)
            y_sb = ep.tile([1, 256], dt, tag="y")
            nc.vector.tensor_scalar_add(y_sb[:], y_ps[:], l2b_t[:])
            nc.sync.dma_start(out=t_y[:], in_=y_sb[:])

    nc.compile()
    return nc


def _get_static(edge_index, batch_index):
    key = hashlib.md5(
        np.ascontiguousarray(edge_index).tobytes()
        + np.ascontiguousarray(batch_index).tobytes()
    ).hexdigest()
    if key not in _CACHE:
        meta = _preprocess(edge_index, batch_index)
        meta['nc'] = _build_nc(meta)
        _CACHE[key] = meta
    return _CACHE[key]


def _build_in_maps(inputs, meta):
    import ml_dtypes
    bf16 = ml_dtypes.bfloat16

    x = np.ascontiguousarray(np.asarray(inputs['x'], np.float32))
    NB, CAP, V = meta['NB'], meta['CAP'], meta['V']
    node_start, g = meta['node_start'], meta['g']

    table1 = np.zeros((V, 64), bf16)
    table1[g] = x.astype(bf16)

    f32 = lambda a, shp: np.ascontiguousarray(np.asarray(a, np.float32).reshape(shp))
    shared = {
        "table1": table1,
        "iota128": np.tile(np.arange(P, dtype=np.float32), (P, 1)).astype(bf16),
        "iota256": np.tile(np.arange(256, dtype=np.float32), (P, 1)),
        "idgw": np.ascontiguousarray(np.concatenate(
            [np.eye(64, dtype=np.float32),
             np.asarray(inputs['gate_w'], np.float32).reshape(64, 1)], axis=1)),
        "ones64": np.ones((1, 64), np.float32),
        "w1l": f32(inputs['w1l'], (64, 64)), "w1r": f32(inputs['w1r'], (64, 64)),
        "w2l": f32(inputs['w2l'], (64, 64)), "w2r": f32(inputs['w2r'], (64, 64)),
        "w3l": f32(inputs['w3l'], (64, 64)), "w3r": f32(inputs['w3r'], (64, 64)),
        "lin1_w": f32(inputs['lin1_w'], (64, 64)),
        "b1l": f32(inputs['b1l'], (64, 1)), "b2l": f32(inputs['b2l'], (64, 1)),
        "b3l": f32(inputs['b3l'], (64, 1)), "lin1_b": f32(inputs['lin1_b'], (64, 1)),
        "gate_b": np.tile(f32(inputs['gate_b'], (1, 1)), (P, 1)),
        "lin2_w": f32(inputs['lin2_w'], (64, 1)),
        "lin2_b": f32(inputs['lin2_b'], (1, 1)),
    }

    in_maps = []
    for c in range(NC):
        M = int(meta['Mc'][c])
        xT = np.zeros((CAP, 64), np.float32)
        xT[:M] = x[node_start[c]:node_start[c + 1]]
        xTb = np.ascontiguousarray(
            xT.reshape(NB, P, 64).transpose(0, 2, 1))   # [NB, 64, 128]
        in_maps.append({
            **shared,
            "xTb": xTb,
            "idx16": meta['idx16'][c],
            "dstrel": meta['dstrel'][c].astype(bf16),
            "invb": meta['invb'][c],
            "brel": meta['brel'][c],
        })
    return in_maps


def kernel(**inputs):
    from concourse.bass_utils import run_bass_kernel_spmd
    meta = _get_static(inputs['edge_index'], inputs['batch_index'])
    in_maps = _build_in_maps(inputs, meta)
    res = run_bass_kernel_spmd(meta['nc'], in_maps, core_ids=list(range(NC)))
    out = np.empty((NG, 1), np.float32)
    for c in range(NC):
        out[c * GPC:(c + 1) * GPC, 0] = res.results[c]["y"][0]
    return out

